# revision 6
# baseline (speedup 1.0000x reference)
"""Trainium2 Bass kernel for nn_MultiHeadedAttention_31095563223751.

Multi-headed attention with an AST-parent-embedding additive score term:
  q,k,v = proj(query/key/value); qk = q @ k^T
  score_ast[b,h,i,j] = q[b,h,i,:] . table(parity j)[ast[b,i,j]]
  attn = softmax(qk + score_ast + mask); out = (attn @ v) @ Wo + bo
Returns (output, attn) like the reference.

Sharding: 8 cores = 4 batches x 2 query-row halves (data parallel); all 16
heads per core.  Each core computes K/V projections for its full batch.

The score_ast gather runs on-device: qsv[h,i,n] = q . table_n is a dense
matmul over the (only 400-entry) combined struct/value table, and the
per-(i,j) scalar gather qsv[h,i,tab[i,j]] runs on the GPSIMD engine via
ap_gather (16 heads share each (b,i) index list -> 16-partition groups).

Row permutation: within each 128-row chunk, device row p holds logical query
row 8*(p%16) + p//16 so the PSUM->gather-layout shuffles become clean DMA
patterns (group g of gather-tile t is device row g*16+t).  The host
un-permutes the returned attn/out rows.
"""

import math

import numpy as np

import concourse.bass as bass
import concourse.tile as tile
import concourse.mybir as mybir
from concourse import bacc
import concourse.bass_utils as bass_utils
from concourse.masks import make_identity

dt = mybir.dt
AOP = mybir.AluOpType
ACT_F = mybir.ActivationFunctionType

B, L, D, H, DH = 4, 512, 1024, 16, 64
N_CORES = 8
IH = L // 2            # query rows per core
NTAB = 200             # entries per table stream (randint bound in the spec)
NN = 2 * NTAB          # combined table size
PAD_NN = 512

ctx_ps_cur = [None]


def _perm_rows():
    """Device row p (within a 128-chunk) holds logical row 8*(p%16)+p//16."""
    p = np.arange(128)
    return 8 * (p % 16) + p // 16


def build_kernel(with_mask: bool, zero_bias: bool):
    nc = bacc.Bacc("TRN2", target_bir_lowering=False, debug=False,
                   num_devices=N_CORES)
    f32, f32r, fp16, i16 = dt.float32, dt.float32r, dt.float16, dt.int16

    # ---- DRAM I/O ----
    xqT_d = nc.dram_tensor("xqT", [D, IH], f32r, kind="ExternalInput").ap()
    xkT_d = nc.dram_tensor("xkT", [D, L], f32r, kind="ExternalInput").ap()
    xvT_d = nc.dram_tensor("xvT", [D, L], f32r, kind="ExternalInput").ap()
    W_d = {w: nc.dram_tensor(w, [D, D], f32r, kind="ExternalInput").ap()
           for w in ("Wq", "Wk", "Wv", "Wo")}
    svt_d = nc.dram_tensor("SVT", [128, PAD_NN], f32r, kind="ExternalInput").ap()
    idx_d = nc.dram_tensor("IDX", [128, 32, 32], i16, kind="ExternalInput").ap()
    if not zero_bias:
        b2_d = {w: nc.dram_tensor(w, [128, 8], f32, kind="ExternalInput").ap()
                for w in ("bq2", "bk2", "bv2")}
    bo_d = nc.dram_tensor("bo_row", [1, D], f32r, kind="ExternalInput").ap()
    if with_mask:
        mrow_d = nc.dram_tensor("maskrow", [1, L], f32r, kind="ExternalInput").ap()
    attn_d = nc.dram_tensor("attn_o", [H, IH, L], f32, kind="ExternalOutput").ap()
    out_d = nc.dram_tensor("out_o", [IH, D], f32, kind="ExternalOutput").ap()
    # DRAM bounce buffers for the partition-crossing shuffles
    bnc1 = nc.dram_tensor("bnc1", [2, H, 128 * NN], f32, kind="Internal").ap()
    bnc2 = nc.dram_tensor("bnc2", [2, H, 128 * L], f32, kind="Internal").ap()

    with tile.TileContext(nc) as tc:
        with tc.tile_pool(name="persist", bufs=1) as pp, \
             tc.tile_pool(name="work", bufs=3) as wk, \
             tc.tile_pool(name="psA", bufs=3, space="PSUM") as psA, \
             tc.tile_pool(name="psB", bufs=2, space="PSUM") as psB, \
             tc.tile_pool(name="psC", bufs=2, space="PSUM") as psC:

            # ---------- persistent small tiles ----------
            ident = pp.tile([128, 128], f32, name="ident")
            make_identity(nc, ident[:])
            ones_f32 = pp.tile([1, 128], f32, name="ones_f32")
            nc.vector.memset(ones_f32[:], 1.0)
            ones_row = pp.tile([1, 128], f32r, name="ones_row")
            nc.vector.tensor_copy(ones_row[:], ones_f32[:])
            svt_s = pp.tile([128, PAD_NN], f32r, name="svt_s")
            nc.sync.dma_start(svt_s[:], svt_d[:])
            idx_s = pp.tile([128, 32, 32], i16, name="idx_s")
            nc.sync.dma_start(idx_s[:], idx_d[:])
            bo_s = pp.tile([1, D], f32r, name="bo_s")
            nc.sync.dma_start(bo_s[:], bo_d[:])
            if with_mask:
                mrow_s = pp.tile([1, L], f32r, name="mrow_s")
                nc.sync.dma_start(mrow_s[:], mrow_d[:])
            b2_s = {}
            if not zero_bias:
                for w in ("bq2", "bk2", "bv2"):
                    b2_s[w] = pp.tile([128, 8], f32, name=w + "_s")
                    nc.sync.dma_start(b2_s[w][:], b2_d[w][:])

            qT_s = pp.tile([128, 8, IH], f32r, name="qT_s")
            kT_s = pp.tile([128, 8, L], f32r, name="kT_s")
            v_s = pp.tile([128, 4, D], fp16, name="v_s")
            ctx_sb = pp.tile([128, 8, IH], f32r, name="ctx_sb")

            # ---------- phase 1: projections ----------
            with tc.tile_pool(name="weights", bufs=2) as wp, \
                 tc.tile_pool(name="xin", bufs=1) as xp:
                xqT_s = xp.tile([128, 8, IH], f32r, name="xqT_s")
                nc.sync.dma_start(xqT_s[:], xqT_d.rearrange("(c p) i -> p c i", p=128))
                xkT_s = xp.tile([128, 8, L], f32r, name="xkT_s")
                nc.sync.dma_start(xkT_s[:], xkT_d.rearrange("(c p) i -> p c i", p=128))
                xvT_s = xp.tile([128, 8, L], f32r, name="xvT_s")
                nc.sync.dma_start(xvT_s[:], xvT_d.rearrange("(c p) i -> p c i", p=128))

                w_tiles = {}
                for w in ("Wq", "Wk", "Wv"):
                    w_tiles[w] = wp.tile([128, 8, D], f32r, name="wt_" + w, tag="wt")
                    nc.sync.dma_start(w_tiles[w][:],
                                      W_d[w].rearrange("(c p) o -> p c o", p=128))

                qscale = 1.0 / math.sqrt(DH)
                # qT[dout, i] = sum_din Wq[din, dout] * xqT[din, i]
                for mc in range(8):
                    ps = psA.tile([128, 512], f32, name="ps_proj", tag="psA")
                    psq = ps[:, :IH]
                    for dc in range(8):
                        nc.tensor.matmul(psq,
                                         w_tiles["Wq"][:, dc, mc * 128:(mc + 1) * 128],
                                         xqT_s[:, dc, :], start=(dc == 0), stop=(dc == 7))
                    if zero_bias:
                        nc.scalar.activation(qT_s[:, mc, :], psq, ACT_F.Copy, scale=qscale)
                    else:
                        nc.vector.tensor_scalar(qT_s[:, mc, :], psq,
                                                b2_s["bq2"][:, mc:mc + 1],
                                                qscale, AOP.add, AOP.mult)
                # kT[dout, j]
                for mc in range(8):
                    ps = psA.tile([128, 512], f32, name="ps_proj", tag="psA")
                    for dc in range(8):
                        nc.tensor.matmul(ps[:],
                                         w_tiles["Wk"][:, dc, mc * 128:(mc + 1) * 128],
                                         xkT_s[:, dc, :], start=(dc == 0), stop=(dc == 7))
                    if zero_bias:
                        nc.any.tensor_copy(kT_s[:, mc, :], ps[:])
                    else:
                        nc.vector.tensor_scalar(kT_s[:, mc, :], ps[:],
                                                b2_s["bk2"][:, mc:mc + 1],
                                                None, AOP.add)
                # v[j, dout] (fp16; bias folded in exactly at the context stage)
                for jc in range(4):
                    for oh in range(2):
                        ps = psA.tile([128, 512], f32, name="ps_proj", tag="psA")
                        for dc in range(8):
                            nc.tensor.matmul(ps[:], xvT_s[:, dc, jc * 128:(jc + 1) * 128],
                                             w_tiles["Wv"][:, dc, oh * 512:(oh + 1) * 512],
                                             start=(dc == 0), stop=(dc == 7))
                        nc.any.tensor_copy(v_s[:, jc, oh * 512:(oh + 1) * 512], ps[:])

            # ---------- phase 2: qsv, gather, attention ----------
            with tc.tile_pool(name="wop", bufs=1) as wop, \
                 tc.tile_pool(name="gio", bufs=1) as gp, \
                 tc.tile_pool(name="attnb", bufs=4) as ab, \
                 tc.tile_pool(name="attnT", bufs=4) as atp, \
                 tc.tile_pool(name="sa", bufs=4) as sap:

                wo_s = wop.tile([128, 8, D], f32r, name="wo_s")
                nc.sync.dma_start(wo_s[:],
                                  W_d["Wo"].rearrange("(c p) o -> p c o", p=128))

                g_in = gp.tile([128, 16, NN], f32, name="g_in")
                g_out = gp.tile([128, 16, L], f32, name="g_out")

                for ch in range(2):
                    # qsv[h][i, n] = sum_dh qT[(h,dh), i] * SVT[dh, n]
                    for h in range(H):
                        hw = h % 2
                        ps = psB.tile([128, NN], f32, name="ps_qsv", tag="psB")
                        nc.tensor.matmul(ps[:],
                                         qT_s[hw * 64:(hw + 1) * 64, h // 2,
                                              ch * 128:(ch + 1) * 128],
                                         svt_s[hw * 64:(hw + 1) * 64, :NN],
                                         start=True, stop=True,
                                         tile_position=(64 * hw, 0))
                        qsv_sb = wk.tile([128, NN], f32, name="qsv_sb", tag="qsvsb")
                        nc.any.tensor_copy(qsv_sb[:], ps[:])
                        nc.sync.dma_start(
                            bnc1[ch, h, :].rearrange("(p n) -> p n", p=128), qsv_sb[:])
                    # one read-back re-interleaves all heads: partition (g,hh)
                    nc.sync.dma_start(
                        g_in[:],
                        bnc1[ch].rearrange("hh (g t n) -> g hh t n", g=8, t=16))
                    for t in range(16):
                        nc.gpsimd.ap_gather(g_out[:, t, :], g_in[:, t, :],
                                            idx_s[:, ch * 16 + t, :],
                                            channels=128, num_elems=NN, d=1, num_idxs=L)
                    nc.sync.dma_start(
                        bnc2[ch].rearrange("hh (g t j) -> g hh t j", g=8, t=16),
                        g_out[:])

                    # attention for this chunk
                    for h in range(H):
                        hw = h % 2
                        hp = h // 2
                        sa_t = sap.tile([128, L], f32, name="sa_t", tag="sa")
                        nc.sync.dma_start(
                            sa_t[:], bnc2[ch, h, :].rearrange("(p j) -> p j", p=128))

                        psqk = psA.tile([128, L], f32, name="ps_qk", tag="psA")
                        nc.tensor.matmul(psqk[:],
                                         qT_s[hw * 64:(hw + 1) * 64, hp,
                                              ch * 128:(ch + 1) * 128],
                                         kT_s[hw * 64:(hw + 1) * 64, hp, :],
                                         start=True, stop=not with_mask,
                                         tile_position=(64 * hw, 0))
                        if with_mask:
                            nc.tensor.matmul(psqk[:], ones_row[:], mrow_s[:],
                                             start=False, stop=True)

                        sc_t = ab.tile([128, L], f32, name="sc_t", tag="sc")
                        nc.vector.tensor_tensor(sc_t[:], psqk[:], sa_t[:], AOP.add)

                        e_t = ab.tile([128, L], f32, name="e_t", tag="et")
                        z_t = wk.tile([128, 1], f32, name="z_t", tag="zt")
                        nc.scalar.activation(e_t[:], sc_t[:], ACT_F.Exp, accum_out=z_t[:])
                        rz_t = wk.tile([128, 1], f32, name="rz_t", tag="rzt")
                        nc.vector.reciprocal(rz_t[:], z_t[:])

                        at_t = ab.tile([128, L], f32, name="at_t", tag="att")
                        nc.scalar.activation(at_t[:], e_t[:], ACT_F.Copy, scale=rz_t[:])
                        nc.sync.dma_start(attn_d[h, ch * 128:(ch + 1) * 128, :], at_t[:])

                        # transpose attn (i<->j) for the AV matmul
                        pst = psC.tile([128, 4, 128], f32, name="ps_t", tag="psC")
                        for jc in range(4):
                            nc.tensor.transpose(pst[:, jc, :],
                                                at_t[:, jc * 128:(jc + 1) * 128],
                                                ident[:])
                        atT = atp.tile([128, 4, 128], fp16, name="atT", tag="atT")
                        nc.any.tensor_copy(atT[:], pst[:])

                        # AV: ctxT[(h,dh), i_ch] += sum_j v[j,(h,dh)] * attnT[j, i_ch]
                        if hw == 0:
                            ctx_ps = psB.tile([128, 128], f32, name="ctx_ps", tag="psB")
                            ctx_ps_cur[0] = ctx_ps
                        else:
                            ctx_ps = ctx_ps_cur[0]
                        for jc in range(4):
                            nc.tensor.matmul(ctx_ps[hw * 64:(hw + 1) * 64, :],
                                             v_s[:, jc, h * 64:(h + 1) * 64],
                                             atT[:, jc, :],
                                             start=(jc == 0), stop=(jc == 3),
                                             tile_position=(0, 64 * hw))
                        if hw == 1:
                            dst = ctx_sb[:, hp, ch * 128:(ch + 1) * 128]
                            if zero_bias:
                                nc.any.tensor_copy(dst, ctx_ps[:])
                            else:
                                nc.vector.tensor_scalar(dst, ctx_ps[:],
                                                        b2_s["bv2"][:, hp:hp + 1],
                                                        None, AOP.add)

                # ---------- output projection ----------
                for ch in range(2):
                    for oh in range(2):
                        ps = psA.tile([128, 512], f32, name="ps_out", tag="psA")
                        for m in range(8):
                            nc.tensor.matmul(ps[:],
                                             ctx_sb[:, m, ch * 128:(ch + 1) * 128],
                                             wo_s[:, m, oh * 512:(oh + 1) * 512],
                                             start=(m == 0), stop=False)
                        nc.tensor.matmul(ps[:], ones_row[:],
                                         bo_s[:, oh * 512:(oh + 1) * 512],
                                         start=False, stop=True)
                        o_t = wk.tile([128, 512], f32, name="o_t", tag="ot")
                        nc.any.tensor_copy(o_t[:], ps[:])
                        nc.sync.dma_start(
                            out_d[ch * 128:(ch + 1) * 128, oh * 512:(oh + 1) * 512],
                            o_t[:])

    nc.compile()
    return nc


def prep_inputs(key, value, query, ast_parents_matrix, mask,
                Wq, bq, Wk, bk, Wv, bv, Wo, bo, struct_emb, value_emb):
    """Build per-core input maps (host-side sharding / layout only)."""
    key = np.asarray(key, np.float32)
    value = np.asarray(value, np.float32)
    query = np.asarray(query, np.float32)
    ast = np.asarray(ast_parents_matrix)
    mask = np.asarray(mask)
    assert ast[:, :, :L].max() < NTAB and ast[:, :, :L].min() >= 0, \
        "table index out of range"

    perm = _perm_rows()
    SVT = np.zeros((128, PAD_NN), np.float32)
    SVT[:DH, :NTAB] = np.ascontiguousarray(np.asarray(struct_emb, np.float32)[:NTAB].T)
    SVT[:DH, NTAB:NN] = np.ascontiguousarray(np.asarray(value_emb, np.float32)[:NTAB].T)
    SVT[64:128] = SVT[0:64]  # replicate for row-group-packed matmuls

    jpar = (np.arange(L) % 2).astype(np.int64) * NTAB  # value-stream offset

    with_mask = bool(mask.any())
    zero_bias = not (np.any(bq) or np.any(bk) or np.any(bv))

    in_maps = []
    meta = []
    for c in range(N_CORES):
        b, ih = divmod(c, 2)
        base = ih * IH
        ilog = np.concatenate([perm, 128 + perm])   # device row -> local row
        rows = base + ilog                          # logical rows in device order
        m = {
            "xqT": np.ascontiguousarray(query[b, rows].T),
            "xkT": np.ascontiguousarray(key[b].T),
            "xvT": np.ascontiguousarray(value[b].T),
            "Wq": np.ascontiguousarray(Wq, np.float32),
            "Wk": np.ascontiguousarray(Wk, np.float32),
            "Wv": np.ascontiguousarray(Wv, np.float32),
            "Wo": np.ascontiguousarray(Wo, np.float32),
            "SVT": SVT,
            "bo_row": np.asarray(bo, np.float32).reshape(1, D),
        }
        # gather index lists: instr (ch,t); group g handles local row ch*128+8t+g
        tab = ast[b, base:base + IH, :L].astype(np.int64) + jpar[None, :]  # (256,512)
        lrow = (np.arange(2)[:, None, None] * 128
                + 8 * np.arange(16)[None, :, None]
                + np.arange(8)[None, None, :])                  # (ch, t, g)
        wrapped = tab[lrow].astype(np.int16).reshape(2, 16, 8, 32, 16)  # ...,(s,hh)
        # idx[g*16+hh, ch*16+t, s] = tab[lrow(ch,t,g)][s*16+hh]
        idx = np.ascontiguousarray(
            wrapped.transpose(2, 4, 0, 1, 3)          # (g, hh, ch, t, s)
            .reshape(128, 32, 32)).astype(np.int16)
        m["IDX"] = idx
        if with_mask:
            m["maskrow"] = np.where(mask[b, 0], np.float32(-1e18),
                                    np.float32(0.0)).reshape(1, L).astype(np.float32)
        if not zero_bias:
            m["bq2"] = np.ascontiguousarray(np.asarray(bq, np.float32).reshape(8, 128).T)
            m["bk2"] = np.ascontiguousarray(np.asarray(bk, np.float32).reshape(8, 128).T)
            m["bv2"] = np.ascontiguousarray(np.asarray(bv, np.float32).reshape(8, 128).T)
        in_maps.append(m)
        meta.append((b, base, rows))
    return in_maps, meta, with_mask, zero_bias


def kernel(key, value, query, ast_parents_matrix, mask,
           Wq, bq, Wk, bk, Wv, bv, Wo, bo, struct_emb, value_emb):
    in_maps, meta, with_mask, zero_bias = prep_inputs(
        key, value, query, ast_parents_matrix, mask,
        Wq, bq, Wk, bk, Wv, bv, Wo, bo, struct_emb, value_emb)

    nc = build_kernel(with_mask, zero_bias)
    res = bass_utils.run_bass_kernel_spmd(nc, in_maps, core_ids=list(range(N_CORES)))

    output = np.zeros((B, L, D), np.float32)
    attn = np.zeros((B, H, L, L), np.float32)
    for c in range(N_CORES):
        b, base, rows = meta[c]
        r = res.results[c]
        attn[b][:, rows, :] = r["attn_o"]
        output[b][rows, :] = r["out_o"]
    return output, attn
